# revision 1
# baseline (speedup 1.0000x reference)
import sys

sys.path.insert(0, "/opt/trn_rl_repo")

import numpy as np
from concourse import bass, bacc, tile, bass_utils
from concourse.bass import mybir

# Problem: queries (8, 2048, 512) f32, items (4096, 512) f32 ->  (8, 2048) f32
#   score = q @ items.T ; j = argmax_m score[t, m] (softmax+top2 reduces to this)
#   out[t] = -score[t, j] / (||q_t|| * ||items_j||)
# Sharding: batch row b -> core b. Per core: T=2048 tokens, M=4096 items, C=512.
# Host precomputes transposes and the O(N*C) norm tables; device does the
# O(T*M*C) scores + argmax-select.

NCORES = 8
T = 2048
C = 512
M = 4096
NT = T // 128   # 16 token tiles
KC = C // 128   # 4 contraction chunks
NB = M // 512   # 8 psum banks of 512 items

F32 = mybir.dt.float32
F32R = mybir.dt.float32r
AX = mybir.AxisListType
OP = mybir.AluOpType


def _build():
    nc = bacc.Bacc()
    qt_d = nc.dram_tensor("qt", [C, T], F32, kind="ExternalInput")
    it_d = nc.dram_tensor("itT", [C, M], F32, kind="ExternalInput")
    n2_d = nc.dram_tensor("n2rep", [128, M], F32, kind="ExternalInput")
    qn2_d = nc.dram_tensor("qn2h", [128, NT], F32, kind="ExternalInput")
    out_d = nc.dram_tensor("out", [128, NT], F32, kind="ExternalOutput")

    qtr = qt_d.bitcast(F32R).rearrange("(a p) t -> p a t", p=128)
    itr = it_d.bitcast(F32R).rearrange("(a p) m -> p a m", p=128)

    with tile.TileContext(nc) as tc:
        with tc.tile_pool(name="big", bufs=1) as big, \
             tc.tile_pool(name="small", bufs=1) as small:

            itemsT = big.tile([128, KC, M], F32R, name="itemsT")
            qT = big.tile([128, KC, T], F32R, name="qT")
            n2rep = big.tile([128, M], F32, name="n2rep")
            qn2 = small.tile([128, NT], F32, name="qn2")

            # DMA order: qT chunk 0 (unblocks tile-0 matmuls), items banks,
            # n2rep (needed by tile-0 mask pass), rest of qT, qn2 (tail only)
            nc.sync.dma_start(out=qT[:, :, 0:512], in_=qtr[:, :, 0:512])
            for b in range(NB):
                nc.sync.dma_start(
                    out=itemsT[:, :, bass.ts(b, 512)],
                    in_=itr[:, :, bass.ts(b, 512)],
                )
            nc.sync.dma_start(out=n2rep, in_=n2_d[:, :])
            for cch in range(1, T // 512):
                nc.sync.dma_start(
                    out=qT[:, :, bass.ts(cch, 512)],
                    in_=qtr[:, :, bass.ts(cch, 512)],
                )
            nc.sync.dma_start(out=qn2, in_=qn2_d[:, :])

            Vs = small.tile([128, NT], F32, name="Vs")
            n2sel = small.tile([128, NT], F32, name="n2sel")
            mask = big.tile([128, M], F32, name="mask")
            with tc.tile_pool(name="bps", bufs=1, space="PSUM") as bps, \
                 tc.tile_pool(name="scp", bufs=2) as scp:
                for i in range(NT):
                    ssb = scp.tile([128, M], F32, tag="ssb", name="ssb")
                    banks = [
                        bps.tile([128, 512], F32, tag=f"bank{b}", name="bank")
                        for b in range(NB)
                    ]
                    # k-outer shares the stationary qT chunk across banks
                    for k in range(KC):
                        for b in range(NB):
                            nc.tensor.matmul(
                                banks[b], qT[:, k, bass.ts(i, 128)],
                                itemsT[:, k, bass.ts(b, 512)],
                                start=(k == 0), stop=(k == KC - 1),
                            )
                    for b in range(NB):
                        nc.scalar.copy(ssb[:, bass.ts(b, 512)], banks[b])
                    # max pass then masked-select pass, both on DVE
                    nc.vector.tensor_reduce(
                        Vs[:, i : i + 1], ssb, axis=AX.X, op=OP.max
                    )
                    nc.vector.scalar_tensor_tensor(
                        out=mask, in0=ssb, scalar=Vs[:, i : i + 1], in1=n2rep,
                        op0=OP.is_ge, op1=OP.mult,
                        accum_out=n2sel[:, i : i + 1],
                    )

            # out = -V / sqrt(qn2 * n2sel)
            prod = small.tile([128, NT], F32, name="prod")
            rcp = small.tile([128, NT], F32, name="rcp")
            outv = small.tile([128, NT], F32, name="outv")
            nc.vector.scalar_tensor_tensor(
                out=prod, in0=qn2, scalar=1.0, in1=n2sel, op0=OP.mult, op1=OP.mult
            )
            nc.scalar.sqrt(prod, prod)
            nc.vector.reciprocal(rcp, prod)
            nc.vector.scalar_tensor_tensor(
                out=outv, in0=Vs, scalar=-1.0, in1=rcp, op0=OP.mult, op1=OP.mult
            )
            nc.sync.dma_start(out=out_d[:, :], in_=outv)

    if not nc.is_finalized():
        nc.finalize()
    return nc


_NC = None


def _run(queries, items, trace=False):
    global _NC
    if _NC is None:
        _NC = _build()
    queries = np.asarray(queries, dtype=np.float32)
    items = np.asarray(items, dtype=np.float32)
    itT = np.ascontiguousarray(items.T)
    i64 = items.astype(np.float64)
    n2 = np.einsum("mc,mc->m", i64, i64).astype(np.float32)
    n2rep = np.ascontiguousarray(np.broadcast_to(n2[None, :], (128, M)))
    in_maps = []
    for b in range(NCORES):
        qb = queries[b]
        q64 = qb.astype(np.float64)
        qn2 = np.einsum("tc,tc->t", q64, q64).astype(np.float32)
        in_maps.append({
            "qt": np.ascontiguousarray(qb.T),
            "itT": itT,
            "n2rep": n2rep,
            "qn2h": np.ascontiguousarray(qn2.reshape(NT, 128).T),
        })
    res = bass_utils.run_bass_kernel_spmd(
        _NC, in_maps, core_ids=list(range(NCORES)), trace=trace
    )
    out = np.stack([r["out"].T.reshape(T) for r in res.results]).astype(np.float32)
    return out, res.exec_time_ns


def kernel(queries, items):
    out, _ = _run(queries, items)
    return out

